# revision 1
# baseline (speedup 1.0000x reference)
"""Trainium2 Bass kernel for nn_AMM_66829691126233 (sparse_attention).

Computation (see reference):
  theta = concat([fm_source*mask_source*0.01, rel_pos_source], ch).reshape(3, 392, 4096)
  phi   = concat([fm_reference*mask_ref*0.01, rel_pos_ref], ch).reshape(3, 392, 4096)
  scores = theta^T @ phi                      (3, 4096, 4096)
  attn = softmax(scores*200, -1) * (scores != 0)
  g3 = (w_lambda . fm_reference) * mask_ref   (3, 4096)
  b3 = (w_beta   . fm_reference) * mask_ref
  gamma = sum_b attn[b] @ g3[b];  beta likewise   (4096,)
  out = fm_source * (1 + gamma) + beta        (1, 256, 64, 64)

Sharding: embarrassingly parallel over query rows. Core k owns pixels
[512k, 512(k+1)) for all 3 parts; each core computes its gamma/beta slice
completely locally (the softmax + both weighted sums are fused, flash-style,
so the 3x4096x4096 score matrix never hits DRAM), then its (256, 512)
output slice. No collectives. Host concatenates the 8 slices.

The (scores != 0) mask is a no-op numerically: any exactly-zero score sits
>= ~7000 logit units below the row max after the x200 scale, so its softmax
weight underflows to 0 in fp32 exactly as the reference's post-softmax mask
would produce.
"""

import sys

if "/opt/trn_rl_repo" not in sys.path:
    sys.path.insert(0, "/opt/trn_rl_repo")

import numpy as np

H = W = 64
HW = H * W          # 4096
C_FM = 256
C_REL = 136
NP = 3
NCORES = 8
S = HW // NCORES    # 512 query pixels per core
NT = S // 128       # 4 query row-tiles per part
import os as _os
MH = int(_os.environ.get("K_MH", "2048"))   # key-dim span per psum tile
NSUB = MH // 512    # 512-wide psum banks per span
NH = HW // MH       # key spans per row-tile
USE_TTR = _os.environ.get("K_TTR", "1") == "1"
DBG = _os.environ.get("K_DBG", "0") == "1"
GPS = _os.environ.get("K_GPS", "0") == "1"   # masking/elementwise prep on gpsimd
SPLIT = _os.environ.get("K_SPLIT", "1") == "1"  # bf16 hi/lo packed rel matmuls

_cache = {}


def _build(debug=False):
    import concourse.bass as bass
    import concourse.tile as tile
    from concourse import bacc, mybir
    from concourse.masks import make_identity

    f32 = mybir.dt.float32
    bf16 = mybir.dt.bfloat16
    AF = mybir.ActivationFunctionType
    ALU = mybir.AluOpType

    nc = bacc.Bacc(None, target_bir_lowering=False, debug=debug)

    fm_src_sl = nc.declare_dram_parameter("fm_src_sl", [C_FM, S], f32, isOutput=False)
    mask_src_sl = nc.declare_dram_parameter("mask_src_sl", [NP, S], f32, isOutput=False)
    rel_src_sl = nc.declare_dram_parameter("rel_src_sl", [NP, C_REL, S], f32, isOutput=False)
    fm_ref = nc.declare_dram_parameter("fm_ref", [C_FM, HW], f32, isOutput=False)
    mask_ref = nc.declare_dram_parameter("mask_ref", [NP, HW], f32, isOutput=False)
    rel_ref = nc.declare_dram_parameter("rel_ref", [NP, C_REL, HW], f32, isOutput=False)
    w_lambda = nc.declare_dram_parameter("w_lambda", [1, C_FM], f32, isOutput=False)
    w_beta = nc.declare_dram_parameter("w_beta", [1, C_FM], f32, isOutput=False)
    out_ext = nc.declare_dram_parameter("out", [C_FM, S], f32, isOutput=True)
    if DBG:
        dbg_ext = nc.declare_dram_parameter("dbg", [NP, 4, 128, NT * NH], f32, isOutput=True)
        dbg2_ext = nc.declare_dram_parameter("dbg2", [2, 128, NP * NT], f32, isOutput=True)

    gb_dram = nc.dram_tensor("gb_scratch", [8, 128], f32)

    with tile.TileContext(nc) as tc:
        with (
            tc.tile_pool(name="static", bufs=1) as st,
            tc.tile_pool(name="parts2", bufs=2) as p2,
            tc.tile_pool(name="parts1", bufs=1) as p1,
            tc.tile_pool(name="work", bufs=3) as wk,
            tc.tile_pool(name="stats", bufs=2) as sp,
            tc.tile_pool(name="psum", bufs=2, space="PSUM") as pm,
            tc.tile_pool(name="dram", bufs=2, space="DRAM") as dp,
        ):
            # ---------------- phase 0: weights, fm_ref prep ----------------
            wlam_row = st.tile([1, C_FM], f32)
            nc.sync.dma_start(out=wlam_row[:], in_=w_lambda[0:1, :])
            wbeta_row = st.tile([1, C_FM], f32)
            nc.sync.dma_start(out=wbeta_row[:], in_=w_beta[0:1, :])
            ones1 = st.tile([1, 128], f32)
            nc.vector.memset(ones1[:], 1.0)

            # fm_ref / fm_src chunks: bf16 via casting DMA; raw fp32 fm_src for output
            fmr_bf = []
            for c in range(2):
                t = st.tile([128, HW], bf16, name=f"fmr_bf{c}")
                nc.gpsimd.dma_start(out=t[:], in_=fm_ref[c * 128:(c + 1) * 128, :])
                fmr_bf.append(t)
            fms_raw = []
            fms_bf = []
            for c in range(2):
                t = st.tile([128, S], f32, name=f"fms_raw{c}")
                nc.sync.dma_start(out=t[:], in_=fm_src_sl[c * 128:(c + 1) * 128, :])
                fms_raw.append(t)
                tb = st.tile([128, S], bf16, name=f"fms_bf{c}")
                nc.gpsimd.dma_start(out=tb[:], in_=fm_src_sl[c * 128:(c + 1) * 128, :])
                fms_bf.append(tb)

            # replicate w_lambda / w_beta chunks across partitions via K=1 matmul
            wrep_bf = []  # [wl0, wl1, wb0, wb1]
            for q, (row, c) in enumerate([(wlam_row, 0), (wlam_row, 1), (wbeta_row, 0), (wbeta_row, 1)]):
                ps_w = pm.tile([128, MH], f32, tag="ps", name=f"ps_w{q}")
                nc.tensor.matmul(
                    ps_w[:, 0:128],
                    row[0:1, c * 128:(c + 1) * 128],
                    ones1[0:1, :],
                    start=True, stop=True,
                )
                t = st.tile([128, 128], bf16, name=f"wrep{q}")
                nc.scalar.copy(t[:], ps_w[:, 0:128])
                wrep_bf.append(t)

            # old_gamma / old_beta replicated on all 128 partitions: [128, HW] bf16
            old_rep = []
            for vi in range(2):
                dst = st.tile([128, HW], bf16, name=f"old_rep{vi}")
                for hh in range(HW // MH):
                    pg = pm.tile([128, MH], f32, tag="ps", name=f"ps_old{vi}{hh}")
                    for k in range(NSUB):
                        col = slice(k * 512, (k + 1) * 512)
                        src = slice(hh * MH + k * 512, hh * MH + (k + 1) * 512)
                        for c in range(2):
                            nc.tensor.matmul(
                                pg[:, col],
                                wrep_bf[2 * vi + c][:],
                                fmr_bf[c][:, src],
                                start=(c == 0), stop=(c == 1),
                            )
                    nc.scalar.copy(dst[:, hh * MH:(hh + 1) * MH], pg[:])
                old_rep.append(dst)

            # scale fm chunks by 0.01 in place (raw bf16 copies are dead after
            # the old_gamma/old_beta matmuls above)
            for c in range(2):
                nc.vector.tensor_scalar_mul(fmr_bf[c][:], fmr_bf[c][:], 0.01)
                nc.vector.tensor_scalar_mul(fms_bf[c][:], fms_bf[c][:], 0.01)
            fmr01, fms01 = fmr_bf, fms_bf

            ident = st.tile([128, 128], f32)
            make_identity(nc, ident[:])

            gacc = st.tile([128, NP * NT], f32)   # per-part gamma partials
            bacc_t = st.tile([128, NP * NT], f32)

            # ---------------- main: parts x row-tiles x key-halves ----------------
            for b in range(NP):
                mask_rep = p2.tile([128, HW], bf16, tag="mask_rep", name=f"mask_rep{b}")
                nc.gpsimd.dma_start(out=mask_rep[:], in_=mask_ref[b:b + 1, :].to_broadcast([128, HW]))
                mask_src_rep = p2.tile([128, S], bf16, tag="mask_src_rep", name=f"msrc_rep{b}")
                nc.gpsimd.dma_start(out=mask_src_rep[:], in_=mask_src_sl[b:b + 1, :].to_broadcast([128, S]))

                eng = nc.gpsimd if GPS else nc.vector
                if SPLIT:
                    # rel channels as packed bf16 hi/lo: score contribution is
                    # hi.hi + hi.lo + lo.hi across chunk pairs (c2..c5):
                    #  c2[0:128] = (th_hi[0:128],  ph_hi[0:128])
                    #  c3[0:96]  = (th_hi[0:96],   ph_lo[0:96]);  c3[96:104] = (th_hi[128:136], ph_hi[128:136])
                    #  c4[0:64]  = (th_lo[0:64],   ph_hi[0:64]);  c4[64:104] = (th_hi[96:136],  ph_lo[96:136])
                    #  c5[0:72]  = (th_lo[64:136], ph_hi[64:136])
                    # lo tensors are built with ONE base-0 subtract over rows
                    # 0..127 plus a cheap flat-column subtract for rows 128..135;
                    # segments then move into packed position via DMA only.
                    raw_ax = p1.tile([128, HW + 256], f32, tag="raw_ax", name=f"raw_ax{b}")
                    nc.sync.dma_start(out=raw_ax[:, 0:HW], in_=rel_ref[b, 0:128, :])
                    nc.sync.dma_start(out=raw_ax[:, HW:HW + 256], in_=rel_ref[b, 128:C_REL, :])
                    P2x = p2.tile([128, HW + 256], bf16, tag="P2x", name=f"P2x{b}")
                    nc.gpsimd.dma_start(out=P2x[:, 0:HW], in_=rel_ref[b, 0:128, :])
                    nc.gpsimd.dma_start(out=P2x[:, HW:HW + 256], in_=rel_ref[b, 128:C_REL, :])

                    P3 = p1.tile([128, HW], bf16, tag="P3", name=f"P3_{b}")
                    eng.tensor_tensor(out=P3[:, :], in0=raw_ax[:, 0:HW], in1=P2x[:, 0:HW],
                                      op=ALU.subtract)
                    tailf = p1.tile([128, 256], bf16, tag="tailf", name=f"tailf{b}")
                    eng.tensor_tensor(out=tailf[:], in0=raw_ax[:, HW:HW + 256],
                                      in1=P2x[:, HW:HW + 256], op=ALU.subtract)
                    taild = dp.tile([8, HW], bf16, tag="taild", name=f"taild{b}")
                    nc.sync.dma_start(out=taild[:], in_=tailf[:])
                    P4 = p1.tile([128, HW], bf16, tag="P4", name=f"P4_{b}")
                    nc.sync.dma_start(out=P4[64:96, :], in_=P3[96:128, :])
                    nc.gpsimd.dma_start(out=P3[96:104, :], in_=rel_ref[b, 128:C_REL, :])
                    nc.sync.dma_start(out=P4[96:104, :], in_=taild[:])
                    nc.gpsimd.dma_start(out=P4[0:64, :], in_=rel_ref[b, 0:64, :])
                    P5 = p1.tile([128, HW], bf16, tag="P5", name=f"P5_{b}")
                    nc.gpsimd.dma_start(out=P5[0:72, :], in_=rel_ref[b, 64:C_REL, :])

                    raw_tax = p1.tile([128, S + 32], f32, tag="raw_tax", name=f"raw_tax{b}")
                    nc.sync.dma_start(out=raw_tax[:, 0:S], in_=rel_src_sl[b, 0:128, :])
                    nc.sync.dma_start(out=raw_tax[:, S:S + 32], in_=rel_src_sl[b, 128:C_REL, :])
                    T2x = p2.tile([128, S + 32], bf16, tag="T2x", name=f"T2x{b}")
                    nc.gpsimd.dma_start(out=T2x[:, 0:S], in_=rel_src_sl[b, 0:128, :])
                    nc.gpsimd.dma_start(out=T2x[:, S:S + 32], in_=rel_src_sl[b, 128:C_REL, :])

                    T4 = p1.tile([128, S], bf16, tag="T4", name=f"T4_{b}")
                    eng.tensor_tensor(out=T4[:, :], in0=raw_tax[:, 0:S], in1=T2x[:, 0:S],
                                      op=ALU.subtract)
                    ttailf = p1.tile([128, 32], bf16, tag="ttailf", name=f"ttailf{b}")
                    eng.tensor_tensor(out=ttailf[:], in0=raw_tax[:, S:S + 32],
                                      in1=T2x[:, S:S + 32], op=ALU.subtract)
                    ttaild = dp.tile([8, S], bf16, tag="ttaild", name=f"ttaild{b}")
                    nc.sync.dma_start(out=ttaild[:], in_=ttailf[:])
                    T5 = p1.tile([128, S], bf16, tag="T5", name=f"T5_{b}")
                    nc.sync.dma_start(out=T5[0:64, :], in_=T4[64:128, :])
                    nc.gpsimd.dma_start(out=T4[64:96, :], in_=rel_src_sl[b, 96:128, :])
                    nc.gpsimd.dma_start(out=T4[96:104, :], in_=rel_src_sl[b, 128:C_REL, :])
                    nc.sync.dma_start(out=T5[64:72, :], in_=ttaild[:])
                    T3 = p1.tile([128, S], bf16, tag="T3", name=f"T3_{b}")
                    nc.gpsimd.dma_start(out=T3[0:96, :], in_=rel_src_sl[b, 0:96, :])
                    nc.gpsimd.dma_start(out=T3[96:104, :], in_=rel_src_sl[b, 128:C_REL, :])
                else:
                    ph_rel_a = p2.tile([128, HW], f32, tag="ph_rel_a", name=f"ph_rel_a{b}")
                    nc.sync.dma_start(out=ph_rel_a[:], in_=rel_ref[b, 0:128, :])
                    ph_rel_b = p2.tile([8, HW], f32, tag="ph_rel_b", name=f"ph_rel_b{b}")
                    nc.sync.dma_start(out=ph_rel_b[:], in_=rel_ref[b, 128:C_REL, :])
                    th_rel_a = p2.tile([128, S], f32, tag="th_rel_a", name=f"th_rel_a{b}")
                    nc.sync.dma_start(out=th_rel_a[:], in_=rel_src_sl[b, 0:128, :])
                    th_rel_b = p2.tile([8, S], f32, tag="th_rel_b", name=f"th_rel_b{b}")
                    nc.sync.dma_start(out=th_rel_b[:], in_=rel_src_sl[b, 128:C_REL, :])

                # masked bf16 fm sides: x * 0.01 * mask
                ph_fm = []
                for c in range(2):
                    t = p1.tile([128, HW], bf16, tag=f"ph_fm{c}", name=f"ph_fm{b}{c}")
                    eng.tensor_tensor(out=t[:], in0=fmr01[c][:], in1=mask_rep[:], op=ALU.mult)
                    ph_fm.append(t)
                th_fm = []
                for c in range(2):
                    t = p1.tile([128, S], bf16, tag=f"th_fm{c}", name=f"th_fm{b}{c}")
                    eng.tensor_tensor(out=t[:], in0=fms01[c][:], in1=mask_src_rep[:], op=ALU.mult)
                    th_fm.append(t)

                w_g = p1.tile([128, HW], bf16, tag="w_g", name=f"w_g{b}")
                eng.tensor_tensor(out=w_g[:], in0=old_rep[0][:], in1=mask_rep[:], op=ALU.mult)
                w_b = p1.tile([128, HW], bf16, tag="w_b", name=f"w_b{b}")
                eng.tensor_tensor(out=w_b[:], in0=old_rep[1][:], in1=mask_rep[:], op=ALU.mult)

                # per-part stats: col = nt*2 + h
                nm_st = sp.tile([128, NT * NH], f32, tag="nm", name=f"nm{b}")   # -max(200*s)
                z_st = sp.tile([128, NT * NH], f32, tag="z", name=f"z{b}")
                ng_st = sp.tile([128, NT * NH], f32, tag="ng", name=f"ng{b}")
                nb_st = sp.tile([128, NT * NH], f32, tag="nb", name=f"nb{b}")

                for nt in range(NT):
                    nsl = slice(nt * 128, (nt + 1) * 128)
                    for h in range(NH):
                        col = nt * NH + h
                        ps = pm.tile([128, MH], f32, tag="ps", name=f"ps{b}{nt}{h}")
                        if SPLIT:
                            chunks = [
                                (th_fm[0][:, nsl], ph_fm[0]),
                                (th_fm[1][:, nsl], ph_fm[1]),
                                (T2x[:, nsl], P2x),
                                (T3[0:104, nsl], P3),
                                (T4[0:104, nsl], P4),
                                (T5[0:72, nsl], P5),
                            ]
                        else:
                            chunks = [
                                (th_fm[0][:, nsl], ph_fm[0]),
                                (th_fm[1][:, nsl], ph_fm[1]),
                                (th_rel_a[:, nsl], ph_rel_a),
                                (th_rel_b[:, nsl], ph_rel_b),
                            ]
                        nchunks = len(chunks)
                        for ci, (lhsT, ph) in enumerate(chunks):
                            rows = lhsT.partition_size()
                            for k in range(NSUB):
                                pcol = slice(k * 512, (k + 1) * 512)
                                msl = slice(h * MH + k * 512, h * MH + (k + 1) * 512)
                                nc.tensor.matmul(ps[:, pcol], lhsT, ph[0:rows, msl],
                                                 start=(ci == 0), stop=(ci == nchunks - 1))

                        # row max of 100*s via fp16 copy (10-bit mantissa: ulp(2e4)=16,
                        # so the exp argument below stays within ~32 of 0), then
                        # e = exp(200*s - 2*max100)
                        s2 = wk.tile([128, MH], mybir.dt.float16, tag="s2", name=f"s2_{b}{nt}{h}", bufs=2)
                        nc.scalar.mul(s2[:], ps[:], 100.0)
                        m1 = wk.tile([128, MH // 2], mybir.dt.float16, tag="m1",
                                     name=f"m1_{b}{nt}{h}", bufs=2)
                        nc.vector.tensor_tensor(out=m1[:], in0=s2[:, 0:MH // 2],
                                                in1=s2[:, MH // 2:MH], op=ALU.max)
                        mx = wk.tile([128, 1], f32, tag="mx", name=f"mx{b}{nt}{h}")
                        nc.vector.tensor_reduce(out=mx[:], in_=m1[:], axis=mybir.AxisListType.X,
                                                op=ALU.max)
                        nc.vector.tensor_scalar_mul(nm_st[:, col:col + 1], mx[:], -2.0)
                        e_t = wk.tile([128, MH], bf16, tag="e", name=f"e{b}{nt}{h}", bufs=3)
                        junk = wk.tile([128, MH], bf16, tag="junk", name=f"junk{b}{nt}{h}", bufs=1)
                        nc.scalar.activation(
                            out=e_t[:], in_=ps[:], func=AF.Exp,
                            bias=nm_st[:, col:col + 1], scale=200.0,
                            accum_out=z_st[:, col:col + 1],
                        )
                        hsl = slice(h * MH, (h + 1) * MH)
                        if USE_TTR:
                            nc.vector.scalar_tensor_tensor(
                                out=junk[:], in0=e_t[:], scalar=1.0, in1=w_g[:, hsl],
                                op0=ALU.mult, op1=ALU.mult, accum_out=ng_st[:, col:col + 1],
                            )
                            nc.vector.scalar_tensor_tensor(
                                out=junk[:], in0=e_t[:], scalar=1.0, in1=w_b[:, hsl],
                                op0=ALU.mult, op1=ALU.mult, accum_out=nb_st[:, col:col + 1],
                            )
                        else:
                            nc.vector.tensor_tensor(out=s2[:], in0=e_t[:], in1=w_g[:, hsl], op=ALU.mult)
                            nc.vector.tensor_reduce(out=ng_st[:, col:col + 1], in_=s2[:],
                                                    axis=mybir.AxisListType.X, op=ALU.add)
                            nc.vector.tensor_tensor(out=s2[:], in0=e_t[:], in1=w_b[:, hsl], op=ALU.mult)
                            nc.vector.tensor_reduce(out=nb_st[:, col:col + 1], in_=s2[:],
                                                    axis=mybir.AxisListType.X, op=ALU.add)

                # combine the two key-halves of each row-tile, then gamma_b = Ng/Z
                nm2 = nm_st[:].rearrange("p (t h) -> p t h", h=NH)
                nmm = sp.tile([128, NT], f32, tag="nmm", name=f"nmm{b}")
                nc.vector.tensor_reduce(out=nmm[:], in_=nm2, axis=mybir.AxisListType.X, op=ALU.min)
                d2 = sp.tile([128, NT, NH], f32, tag="d2", name=f"d2{b}")
                for h in range(NH):
                    nc.vector.tensor_tensor(out=d2[:, :, h], in0=nmm[:], in1=nm2[:, :, h],
                                            op=ALU.subtract)
                c2 = sp.tile([128, NT, NH], f32, tag="c2", name=f"c2{b}")
                nc.scalar.activation(out=c2[:], in_=d2[:], func=AF.Exp)
                if DBG:
                    nc.sync.dma_start(out=dbg_ext[b, 0], in_=nm_st[:])
                    nc.sync.dma_start(out=dbg_ext[b, 1], in_=z_st[:])
                    nc.sync.dma_start(out=dbg_ext[b, 2], in_=ng_st[:])
                    nc.sync.dma_start(out=dbg_ext[b, 3], in_=nb_st[:])
                for name, stt, acc in (("z", z_st, None), ("g", ng_st, gacc), ("bb", nb_st, bacc_t)):
                    sc = sp.tile([128, NT, NH], f32, tag=f"sc_{name}", name=f"sc_{name}{b}")
                    nc.vector.tensor_tensor(out=sc[:], in0=stt[:].rearrange("p (t h) -> p t h", h=NH),
                                            in1=c2[:], op=ALU.mult)
                    if name == "z":
                        zi = sp.tile([128, NT], f32, tag="zi", name=f"zi{b}")
                        nc.vector.tensor_reduce(out=zi[:], in_=sc[:], axis=mybir.AxisListType.X,
                                                op=ALU.add)
                        rz = sp.tile([128, NT], f32, tag="rz", name=f"rz{b}")
                        nc.vector.reciprocal(rz[:], zi[:])
                    else:
                        si = sp.tile([128, NT], f32, tag=f"si_{name}", name=f"si_{name}{b}")
                        nc.vector.tensor_reduce(out=si[:], in_=sc[:], axis=mybir.AxisListType.X,
                                                op=ALU.add)
                        nc.vector.tensor_tensor(out=acc[:, b * NT:(b + 1) * NT], in0=si[:],
                                                in1=rz[:], op=ALU.mult)

            if DBG:
                nc.sync.dma_start(out=dbg2_ext[0], in_=gacc[:])
                nc.sync.dma_start(out=dbg2_ext[1], in_=bacc_t[:])
            # ---------------- epilogue: gamma/beta assembly + output ----------------
            gb_sb = st.tile([128, 8], f32)
            nc.vector.tensor_reduce(out=gb_sb[:, 0:NT],
                                    in_=gacc[:].rearrange("p (b t) -> p t b", b=NP),
                                    axis=mybir.AxisListType.X, op=ALU.add)
            nc.vector.tensor_scalar_add(gb_sb[:, 0:NT], gb_sb[:, 0:NT], 1.0)
            nc.vector.tensor_reduce(out=gb_sb[:, NT:8],
                                    in_=bacc_t[:].rearrange("p (b t) -> p t b", b=NP),
                                    axis=mybir.AxisListType.X, op=ALU.add)

            ps_t = pm.tile([128, MH], f32, tag="ps", name="ps_tr")
            nc.tensor.transpose(ps_t[:8, 0:128], gb_sb[:], ident[:])
            gb_t = st.tile([8, 128], f32)
            nc.scalar.copy(gb_t[:], ps_t[:8, 0:128])
            nc.sync.dma_start(out=gb_dram[:], in_=gb_t[:])

            g1_rep = st.tile([128, S], f32)
            nc.sync.dma_start(out=g1_rep[:],
                              in_=gb_dram[0:NT, :].unsqueeze(0).to_broadcast([128, NT, 128]))
            b_rep = st.tile([128, S], f32)
            nc.sync.dma_start(out=b_rep[:],
                              in_=gb_dram[NT:8, :].unsqueeze(0).to_broadcast([128, NT, 128]))

            for c in range(2):
                o_t = wk.tile([128, S], f32, tag="o", name=f"o{c}", bufs=2)
                nc.vector.tensor_tensor(out=o_t[:], in0=fms_raw[c][:], in1=g1_rep[:], op=ALU.mult)
                nc.vector.tensor_tensor(out=o_t[:], in0=o_t[:], in1=b_rep[:], op=ALU.add)
                nc.sync.dma_start(out=out_ext[c * 128:(c + 1) * 128, :], in_=o_t[:])

    nc.compile()
    return nc


def kernel(fm_source, fm_reference, mask_source, mask_ref,
           rel_pos_source, rel_pos_ref, w_lambda, w_beta):
    from concourse.bass_utils import run_bass_kernel_spmd

    if "nc" not in _cache:
        _cache["nc"] = _build()
    nc = _cache["nc"]

    fm_src = np.ascontiguousarray(np.asarray(fm_source, np.float32).reshape(C_FM, HW))
    fm_refm = np.ascontiguousarray(np.asarray(fm_reference, np.float32).reshape(C_FM, HW))
    m_src = np.ascontiguousarray(np.asarray(mask_source, np.float32).reshape(NP, HW))
    m_ref = np.ascontiguousarray(np.asarray(mask_ref, np.float32).reshape(NP, HW))
    r_src = np.ascontiguousarray(np.asarray(rel_pos_source, np.float32).reshape(NP, C_REL, HW))
    r_ref = np.ascontiguousarray(np.asarray(rel_pos_ref, np.float32).reshape(NP, C_REL, HW))
    w_l = np.ascontiguousarray(np.asarray(w_lambda, np.float32).reshape(1, C_FM))
    w_b = np.ascontiguousarray(np.asarray(w_beta, np.float32).reshape(1, C_FM))

    in_maps = []
    for k in range(NCORES):
        sl = slice(k * S, (k + 1) * S)
        in_maps.append({
            "fm_src_sl": np.ascontiguousarray(fm_src[:, sl]),
            "mask_src_sl": np.ascontiguousarray(m_src[:, sl]),
            "rel_src_sl": np.ascontiguousarray(r_src[:, :, sl]),
            "fm_ref": fm_refm,
            "mask_ref": m_ref,
            "rel_ref": r_ref,
            "w_lambda": w_l,
            "w_beta": w_b,
        })

    res = run_bass_kernel_spmd(nc, in_maps, list(range(NCORES)))
    _cache["last_result"] = res

    out = np.concatenate([res.results[k]["out"] for k in range(NCORES)], axis=1)
    return out.reshape(1, C_FM, H, W).astype(np.float32)



# revision 9
# speedup vs baseline: 1.0422x; 1.0422x over previous
"""Trainium2 Bass kernel for nn_AMM_66829691126233 (sparse_attention).

Computation (see reference):
  theta = concat([fm_source*mask_source*0.01, rel_pos_source], ch).reshape(3, 392, 4096)
  phi   = concat([fm_reference*mask_ref*0.01, rel_pos_ref], ch).reshape(3, 392, 4096)
  scores = theta^T @ phi                      (3, 4096, 4096)
  attn = softmax(scores*200, -1) * (scores != 0)
  g3 = (w_lambda . fm_reference) * mask_ref   (3, 4096)
  b3 = (w_beta   . fm_reference) * mask_ref
  gamma = sum_b attn[b] @ g3[b];  beta likewise   (4096,)
  out = fm_source * (1 + gamma) + beta        (1, 256, 64, 64)

Sharding: embarrassingly parallel over query rows; core k owns pixels
[512k, 512(k+1)). Flash-style fused softmax+weighted sums; the 3x4096x4096
score matrix never leaves PSUM. No collectives.

The (scores != 0) mask is a numerical no-op (exact zeros sit >=7000 logits
below the row max after the x200 scale; their softmax weight underflows to 0).

v2 layout: the phi-side hi/lo-split tensors are built per (part, key-half)
with bufs=2 tile rotation, and prep for step s+1 is emitted before the
matmul tiles of step s, so DMA/DVE prep overlaps the PE main loop and the
PE never idles long enough for HAM to re-throttle. The two weighted sums
are split by column between DVE and GPSIMD (K_SG).

Precision scheme (unchanged from v1): rel channels as bf16 hi/lo packed
pairs; score contribution = hi.hi + hi.lo + lo.hi over chunks c2..c5:
  c2[0:128] = (th_hi[0:128],  ph_hi[0:128])
  c3[0:96]  = (th_hi[0:96],   ph_lo[0:96]);  c3[96:104] = (th_hi[128:136], ph_hi[128:136])
  c4[0:64]  = (th_lo[0:64],   ph_hi[0:64]);  c4[64:104] = (th_hi[96:136],  ph_lo[96:136])
  c5[0:72]  = (th_lo[64:136], ph_hi[64:136])
fm chunks (x0.01, masked) ride as plain bf16.
"""

import sys

if "/opt/trn_rl_repo" not in sys.path:
    sys.path.insert(0, "/opt/trn_rl_repo")

import os as _os

import numpy as np

H = W = 64
HW = H * W          # 4096
C_FM = 256
C_REL = 136
NP = 3
NCORES = 8
S = HW // NCORES    # 512 query pixels per core
NT = S // 128       # 4 query row-tiles per part
MH = 2048           # key-dim span per psum tile (half of HW)
NSUB = MH // 512    # 512-wide psum banks per span
NH = HW // MH       # key spans (halves) per row-tile

TTR2 = _os.environ.get("K_TTR2", "1") == "1"  # tensor_tensor_reduce for weighted sums
GSUB = _os.environ.get("K_GSUB", "1") == "1"  # hi/lo subtracts on gpsimd
GWGB = _os.environ.get("K_GWGB", "1") == "1"  # w_g/w_b mask mults on gpsimd

_cache = {}


def _build(debug=False):
    import concourse.bass as bass
    import concourse.tile as tile
    from concourse import bacc, mybir
    from concourse.masks import make_identity

    f32 = mybir.dt.float32
    f16 = mybir.dt.float16
    bf16 = mybir.dt.bfloat16
    AF = mybir.ActivationFunctionType
    ALU = mybir.AluOpType

    nc = bacc.Bacc(None, target_bir_lowering=False, debug=debug)

    fm_src_sl = nc.declare_dram_parameter("fm_src_sl", [C_FM, S], f32, isOutput=False)
    mask_src_sl = nc.declare_dram_parameter("mask_src_sl", [NP, S], f32, isOutput=False)
    rel_src_sl = nc.declare_dram_parameter("rel_src_sl", [NP, C_REL, S], f32, isOutput=False)
    fm_ref = nc.declare_dram_parameter("fm_ref", [C_FM, HW], f32, isOutput=False)
    mask_ref = nc.declare_dram_parameter("mask_ref", [NP, HW], f32, isOutput=False)
    rel_ref = nc.declare_dram_parameter("rel_ref", [NP, C_REL, HW], f32, isOutput=False)
    w_lambda = nc.declare_dram_parameter("w_lambda", [1, C_FM], f32, isOutput=False)
    w_beta = nc.declare_dram_parameter("w_beta", [1, C_FM], f32, isOutput=False)
    out_ext = nc.declare_dram_parameter("out", [C_FM, S], f32, isOutput=True)

    gb_dram = nc.dram_tensor("gb_scratch", [8, 128], f32)

    with tile.TileContext(nc) as tc:
        with (
            tc.tile_pool(name="static", bufs=1) as st,
            tc.tile_pool(name="perpart", bufs=2) as pp,
            tc.tile_pool(name="perhalf", bufs=2) as hh,
            tc.tile_pool(name="work", bufs=1) as wk,
            tc.tile_pool(name="stats", bufs=1) as sp,
            tc.tile_pool(name="psum", bufs=2, space="PSUM") as pm,
            tc.tile_pool(name="dram", bufs=2, space="DRAM") as dp,
        ):
            # ---------------- phase 0: weights, fm prep ----------------
            wlam_row = st.tile([1, C_FM], f32)
            nc.sync.dma_start(out=wlam_row[:], in_=w_lambda[0:1, :])
            wbeta_row = st.tile([1, C_FM], f32)
            nc.sync.dma_start(out=wbeta_row[:], in_=w_beta[0:1, :])
            ones1 = st.tile([1, 128], f32)
            nc.vector.memset(ones1[:], 1.0)

            fmr_bf = []
            for c in range(2):
                t = st.tile([128, HW], bf16, name=f"fmr_bf{c}")
                nc.gpsimd.dma_start(out=t[:], in_=fm_ref[c * 128:(c + 1) * 128, :])
                fmr_bf.append(t)
            fms_raw = []
            fms_bf = []
            for c in range(2):
                t = st.tile([128, S], f32, name=f"fms_raw{c}")
                nc.sync.dma_start(out=t[:], in_=fm_src_sl[c * 128:(c + 1) * 128, :])
                fms_raw.append(t)
                tb = st.tile([128, S], bf16, name=f"fms_bf{c}")
                nc.gpsimd.dma_start(out=tb[:], in_=fm_src_sl[c * 128:(c + 1) * 128, :])
                fms_bf.append(tb)

            # replicate w_lambda / w_beta chunks across partitions via K=1 matmul
            wrep_bf = []  # [wl0, wl1, wb0, wb1]
            for q, (row, c) in enumerate([(wlam_row, 0), (wlam_row, 1), (wbeta_row, 0), (wbeta_row, 1)]):
                ps_w = pm.tile([128, MH], f32, tag="ps", name=f"ps_w{q}")
                nc.tensor.matmul(
                    ps_w[:, 0:128],
                    row[0:1, c * 128:(c + 1) * 128],
                    ones1[0:1, :],
                    start=True, stop=True,
                )
                t = st.tile([128, 128], bf16, name=f"wrep{q}")
                nc.scalar.copy(t[:], ps_w[:, 0:128])
                wrep_bf.append(t)

            # old_gamma / old_beta replicated on all 128 partitions: [128, HW] bf16
            old_rep = []
            for vi in range(2):
                dst = st.tile([128, HW], bf16, name=f"old_rep{vi}")
                for hhh in range(NH):
                    pg = pm.tile([128, MH], f32, tag="ps", name=f"ps_old{vi}{hhh}")
                    for k in range(NSUB):
                        col = slice(k * 512, (k + 1) * 512)
                        src = slice(hhh * MH + k * 512, hhh * MH + (k + 1) * 512)
                        for c in range(2):
                            nc.tensor.matmul(
                                pg[:, col],
                                wrep_bf[2 * vi + c][:],
                                fmr_bf[c][:, src],
                                start=(c == 0), stop=(c == 1),
                            )
                    nc.scalar.copy(dst[:, hhh * MH:(hhh + 1) * MH], pg[:])
                old_rep.append(dst)

            # scale fm chunks by 0.01 in place (raw bf16 copies are dead after
            # the old_gamma/old_beta matmuls above)
            for c in range(2):
                nc.vector.tensor_scalar_mul(fmr_bf[c][:], fmr_bf[c][:], 0.01)
                nc.vector.tensor_scalar_mul(fms_bf[c][:], fms_bf[c][:], 0.01)
            fmr01, fms01 = fmr_bf, fms_bf

            ident = st.tile([128, 128], f32)
            make_identity(nc, ident[:])

            # persistent per-part stats (col = nt*NH + h)
            stats = []
            for b in range(NP):
                stats.append({
                    k: sp.tile([128, NT * NH], f32, tag=f"{k}{b}", name=f"{k}{b}")
                    for k in ("nm", "z", "ng", "nb")
                })

            # shared junk output for the accumulating weighted sums
            junk_d = wk.tile([128, MH], bf16, tag="junk_d", name="junk_d")

            # ---------------- prep emitters ----------------
            TP = {}  # per-part theta-side tensors
            PH = {}  # per-(part, half) phi-side tensors

            def part_prep(b):
                """theta-side (query) tensors + masks for part b."""
                d = {}
                mrep = pp.tile([128, HW], bf16, tag="mask_rep", name=f"mask_rep{b}")
                nc.gpsimd.dma_start(out=mrep[:], in_=mask_ref[b:b + 1, :].to_broadcast([128, HW]))
                d["mask"] = mrep
                msrc = pp.tile([128, S], bf16, tag="msrc_rep", name=f"msrc_rep{b}")
                nc.gpsimd.dma_start(out=msrc[:], in_=mask_src_sl[b:b + 1, :].to_broadcast([128, S]))

                raw_t = pp.tile([128, S + 32], f32, tag="raw_tax", bufs=1, name=f"raw_tax{b}")
                nc.sync.dma_start(out=raw_t[:, 0:S], in_=rel_src_sl[b, 0:128, :])
                nc.sync.dma_start(out=raw_t[:, S:S + 32], in_=rel_src_sl[b, 128:C_REL, :])
                T2x = pp.tile([128, S + 32], bf16, tag="T2x", name=f"T2x{b}")
                nc.gpsimd.dma_start(out=T2x[:, 0:S], in_=rel_src_sl[b, 0:128, :])
                nc.gpsimd.dma_start(out=T2x[:, S:S + 32], in_=rel_src_sl[b, 128:C_REL, :])

                sub_eng = nc.gpsimd if GSUB else nc.vector
                T4 = pp.tile([128, S], bf16, tag="T4", name=f"T4_{b}")
                sub_eng.tensor_tensor(out=T4[:, :], in0=raw_t[:, 0:S], in1=T2x[:, 0:S],
                                      op=ALU.subtract)
                ttailf = pp.tile([128, 32], bf16, tag="ttailf", bufs=1, name=f"ttailf{b}")
                sub_eng.tensor_tensor(out=ttailf[:], in0=raw_t[:, S:S + 32],
                                      in1=T2x[:, S:S + 32], op=ALU.subtract)
                ttaild = dp.tile([8, S], bf16, tag="ttaild", name=f"ttaild{b}")
                nc.sync.dma_start(out=ttaild[:], in_=ttailf[:])
                T5 = pp.tile([128, S], bf16, tag="T5", name=f"T5_{b}")
                nc.sync.dma_start(out=T5[0:64, :], in_=T4[64:128, :])
                nc.sync.dma_start(out=T4[64:96, :], in_=T2x[96:128, 0:S])
                nc.gpsimd.dma_start(out=T4[96:104, :], in_=rel_src_sl[b, 128:C_REL, :])
                nc.sync.dma_start(out=T5[64:72, :], in_=ttaild[:])
                T3 = pp.tile([128, S], bf16, tag="T3", name=f"T3_{b}")
                nc.sync.dma_start(out=T3[0:96, :], in_=T2x[0:96, 0:S])
                nc.gpsimd.dma_start(out=T3[96:104, :], in_=rel_src_sl[b, 128:C_REL, :])

                th_fm = []
                for c in range(2):
                    t = pp.tile([128, S], bf16, tag=f"th_fm{c}", name=f"th_fm{b}{c}")
                    nc.vector.tensor_tensor(out=t[:], in0=fms01[c][:], in1=msrc[:], op=ALU.mult)
                    th_fm.append(t)
                d.update(T2x=T2x, T3=T3, T4=T4, T5=T5, th_fm=th_fm)
                TP[b] = d

            def half_prep(b, h):
                """phi-side (key) tensors for part b, key-half h."""
                hsl = slice(h * MH, (h + 1) * MH)
                mrep = TP[b]["mask"]
                raw = hh.tile([128, MH + 128], f32, tag="raw_ax", bufs=1, name=f"raw{b}{h}")
                nc.sync.dma_start(out=raw[:, 0:MH], in_=rel_ref[b, 0:128, hsl])
                nc.sync.dma_start(out=raw[:, MH:MH + 128], in_=rel_ref[b, 128:C_REL, hsl])
                P2x = hh.tile([128, MH], bf16, tag="P2x", name=f"P2x{b}{h}")
                nc.gpsimd.dma_start(out=P2x[:], in_=rel_ref[b, 0:128, hsl])
                P2t = hh.tile([128, 128], bf16, tag="P2t", name=f"P2t{b}{h}")
                nc.gpsimd.dma_start(out=P2t[:], in_=rel_ref[b, 128:C_REL, hsl])

                sub_eng = nc.gpsimd if GSUB else nc.vector
                P3 = hh.tile([128, MH], bf16, tag="P3", name=f"P3_{b}{h}")
                sub_eng.tensor_tensor(out=P3[:, :], in0=raw[:, 0:MH], in1=P2x[:], op=ALU.subtract)
                tailf = hh.tile([128, 128], bf16, tag="tailf", bufs=1, name=f"tailf{b}{h}")
                sub_eng.tensor_tensor(out=tailf[:], in0=raw[:, MH:MH + 128], in1=P2t[:],
                                      op=ALU.subtract)
                taild = dp.tile([8, MH], bf16, tag="taild", name=f"taild{b}{h}")
                nc.sync.dma_start(out=taild[:], in_=tailf[:])
                P4 = hh.tile([128, MH], bf16, tag="P4", name=f"P4_{b}{h}")
                nc.sync.dma_start(out=P4[64:96, :], in_=P3[96:128, :])
                nc.gpsimd.dma_start(out=P3[96:104, :], in_=rel_ref[b, 128:C_REL, hsl])
                nc.sync.dma_start(out=P4[96:104, :], in_=taild[:])
                nc.sync.dma_start(out=P4[0:64, :], in_=P2x[0:64, :])
                P5 = hh.tile([128, MH], bf16, tag="P5", name=f"P5_{b}{h}")
                nc.sync.dma_start(out=P5[0:64, :], in_=P2x[64:128, :])
                nc.gpsimd.dma_start(out=P5[64:72, :], in_=rel_ref[b, 128:C_REL, hsl])

                ph_fm = []
                for c in range(2):
                    t = hh.tile([128, MH], bf16, tag=f"ph_fm{c}", name=f"ph_fm{b}{h}{c}")
                    nc.vector.tensor_tensor(out=t[:], in0=fmr01[c][:, hsl], in1=mrep[:, hsl],
                                            op=ALU.mult)
                    ph_fm.append(t)
                w_eng = nc.gpsimd if GWGB else nc.vector
                w_g = hh.tile([128, MH], bf16, tag="w_g", name=f"w_g{b}{h}")
                w_eng.tensor_tensor(out=w_g[:], in0=old_rep[0][:, hsl], in1=mrep[:, hsl],
                                    op=ALU.mult)
                w_b = hh.tile([128, MH], bf16, tag="w_b", name=f"w_b{b}{h}")
                w_eng.tensor_tensor(out=w_b[:], in0=old_rep[1][:, hsl], in1=mrep[:, hsl],
                                    op=ALU.mult)
                PH[(b, h)] = dict(P2x=P2x, P3=P3, P4=P4, P5=P5, ph_fm=ph_fm, w_g=w_g, w_b=w_b)

            # ---------------- main tile ----------------
            def emit_tile(b, h, nt):
                t = TP[b]
                p = PH[(b, h)]
                stt = stats[b]
                nsl = slice(nt * 128, (nt + 1) * 128)
                col = nt * NH + h
                ps = pm.tile([128, MH], f32, tag="ps", name=f"ps{b}{h}{nt}")
                chunks = [
                    (t["T2x"][0:128, nsl], p["P2x"], 128),
                    (t["T3"][0:104, nsl], p["P3"], 104),
                    (t["T4"][0:104, nsl], p["P4"], 104),
                    (t["T5"][0:72, nsl], p["P5"], 72),
                    (t["th_fm"][0][:, nsl], p["ph_fm"][0], 128),
                    (t["th_fm"][1][:, nsl], p["ph_fm"][1], 128),
                ]
                nchunks = len(chunks)
                for ci, (lhsT, ph, rows) in enumerate(chunks):
                    for k in range(NSUB):
                        pcol = slice(k * 512, (k + 1) * 512)
                        nc.tensor.matmul(ps[:, pcol], lhsT, ph[0:rows, pcol],
                                         start=(ci == 0), stop=(ci == nchunks - 1))

                # row max of 100*s via fp16 copy (10-bit mantissa: ulp(2e4)=16,
                # so the exp argument below stays within ~32 of 0), then
                # e = exp(200*s - 2*max100)
                s2 = wk.tile([128, MH], f16, tag="s2", name=f"s2_{b}{h}{nt}", bufs=2)
                nc.scalar.mul(s2[:], ps[:], 100.0)
                m1 = wk.tile([128, MH // 2], f16, tag="m1", name=f"m1_{b}{h}{nt}", bufs=2)
                nc.vector.tensor_tensor(out=m1[:], in0=s2[:, 0:MH // 2], in1=s2[:, MH // 2:MH],
                                        op=ALU.max)
                m2 = wk.tile([128, MH // 4], f16, tag="m2", name=f"m2_{b}{h}{nt}", bufs=1)
                nc.vector.tensor_tensor(out=m2[:], in0=m1[:, 0:MH // 4], in1=m1[:, MH // 4:MH // 2],
                                        op=ALU.max)
                m3 = wk.tile([128, MH // 8], f16, tag="m3", name=f"m3_{b}{h}{nt}", bufs=1)
                nc.vector.tensor_tensor(out=m3[:], in0=m2[:, 0:MH // 8], in1=m2[:, MH // 8:MH // 4],
                                        op=ALU.max)
                mx = wk.tile([128, 1], f32, tag="mx", name=f"mx{b}{h}{nt}", bufs=2)
                nc.vector.tensor_reduce(out=mx[:], in_=m3[:], axis=mybir.AxisListType.X, op=ALU.max)
                nc.vector.tensor_scalar_mul(stt["nm"][:, col:col + 1], mx[:], -2.0)

                e_t = wk.tile([128, MH], bf16, tag="e", name=f"e{b}{h}{nt}", bufs=2)
                nc.scalar.activation(
                    out=e_t[:], in_=ps[:], func=AF.Exp,
                    bias=stt["nm"][:, col:col + 1], scale=200.0,
                    accum_out=stt["z"][:, col:col + 1],
                )
                # weighted sums on DVE
                for key, wvec in (("g", p["w_g"]), ("b", p["w_b"])):
                    if TTR2:
                        nc.vector.tensor_tensor_reduce(
                            out=junk_d[:], in0=e_t[:], in1=wvec[:],
                            scale=1.0, scalar=0.0, op0=ALU.mult, op1=ALU.add,
                            accum_out=stt[f"n{key}"][:, col:col + 1],
                        )
                    else:
                        nc.vector.scalar_tensor_tensor(
                            out=junk_d[:], in0=e_t[:], scalar=1.0,
                            in1=wvec[:], op0=ALU.mult, op1=ALU.mult,
                            accum_out=stt[f"n{key}"][:, col:col + 1],
                        )

            # ---------------- schedule ----------------
            steps = [(b, h) for b in range(NP) for h in range(NH)]
            part_prep(0)
            half_prep(0, 0)
            half_prep(0, 1)
            for si, (b, h) in enumerate(steps):
                for nt in range(NT):
                    emit_tile(b, h, nt)
                # emit prep two halves ahead
                ni = si + 2
                if ni < len(steps):
                    nb_, nh_ = steps[ni]
                    if nh_ == 0:
                        part_prep(nb_)
                    half_prep(nb_, nh_)

            # ---------------- epilogue: combine stats, assemble output ----------------
            gacc = st.tile([128, NP * NT], f32)
            bacc_t = st.tile([128, NP * NT], f32)
            for b in range(NP):
                stt = stats[b]
                nm2 = stt["nm"][:].rearrange("p (t h) -> p t h", h=NH)
                nmm = sp.tile([128, NT], f32, tag=f"nmm{b}", name=f"nmm{b}")
                nc.vector.tensor_reduce(out=nmm[:], in_=nm2, axis=mybir.AxisListType.X, op=ALU.min)
                d2 = sp.tile([128, NT, NH], f32, tag=f"d2{b}", name=f"d2{b}")
                for h in range(NH):
                    nc.vector.tensor_tensor(out=d2[:, :, h], in0=nmm[:], in1=nm2[:, :, h],
                                            op=ALU.subtract)
                c2 = sp.tile([128, NT, NH], f32, tag=f"c2{b}", name=f"c2{b}")
                nc.scalar.activation(out=c2[:], in_=d2[:], func=AF.Exp)
                for name, s1, acc in (("z", "z", None),
                                      ("g", "ng", gacc),
                                      ("bb", "nb", bacc_t)):
                    tot = stt[s1]
                    sc = sp.tile([128, NT, NH], f32, tag=f"sc_{name}{b}", name=f"sc_{name}{b}")
                    nc.vector.tensor_tensor(out=sc[:], in0=tot[:].rearrange("p (t h) -> p t h", h=NH),
                                            in1=c2[:], op=ALU.mult)
                    if name == "z":
                        zi = sp.tile([128, NT], f32, tag=f"zi{b}", name=f"zi{b}")
                        nc.vector.tensor_reduce(out=zi[:], in_=sc[:], axis=mybir.AxisListType.X,
                                                op=ALU.add)
                        rz = sp.tile([128, NT], f32, tag=f"rz{b}", name=f"rz{b}")
                        nc.vector.reciprocal(rz[:], zi[:])
                    else:
                        si_t = sp.tile([128, NT], f32, tag=f"si_{name}{b}", name=f"si_{name}{b}")
                        nc.vector.tensor_reduce(out=si_t[:], in_=sc[:], axis=mybir.AxisListType.X,
                                                op=ALU.add)
                        nc.vector.tensor_tensor(out=acc[:, b * NT:(b + 1) * NT], in0=si_t[:],
                                                in1=rz[:], op=ALU.mult)

            gb_sb = st.tile([128, 8], f32)
            nc.vector.tensor_reduce(out=gb_sb[:, 0:NT],
                                    in_=gacc[:].rearrange("p (b t) -> p t b", b=NP),
                                    axis=mybir.AxisListType.X, op=ALU.add)
            nc.vector.tensor_scalar_add(gb_sb[:, 0:NT], gb_sb[:, 0:NT], 1.0)
            nc.vector.tensor_reduce(out=gb_sb[:, NT:8],
                                    in_=bacc_t[:].rearrange("p (b t) -> p t b", b=NP),
                                    axis=mybir.AxisListType.X, op=ALU.add)

            ps_t = pm.tile([128, MH], f32, tag="ps", name="ps_tr")
            nc.tensor.transpose(ps_t[:8, 0:128], gb_sb[:], ident[:])
            gb_t = st.tile([8, 128], f32)
            nc.scalar.copy(gb_t[:], ps_t[:8, 0:128])
            nc.sync.dma_start(out=gb_dram[:], in_=gb_t[:])

            g1_rep = st.tile([128, S], f32)
            nc.sync.dma_start(out=g1_rep[:],
                              in_=gb_dram[0:NT, :].unsqueeze(0).to_broadcast([128, NT, 128]))
            b_rep = st.tile([128, S], f32)
            nc.sync.dma_start(out=b_rep[:],
                              in_=gb_dram[NT:8, :].unsqueeze(0).to_broadcast([128, NT, 128]))

            for c in range(2):
                o_t = wk.tile([128, S], f32, tag="o", name=f"o{c}", bufs=2)
                nc.vector.tensor_tensor(out=o_t[:], in0=fms_raw[c][:], in1=g1_rep[:], op=ALU.mult)
                nc.vector.tensor_tensor(out=o_t[:], in0=o_t[:], in1=b_rep[:], op=ALU.add)
                nc.sync.dma_start(out=out_ext[c * 128:(c + 1) * 128, :], in_=o_t[:])

    nc.compile()
    return nc


def kernel(fm_source, fm_reference, mask_source, mask_ref,
           rel_pos_source, rel_pos_ref, w_lambda, w_beta):
    from concourse.bass_utils import run_bass_kernel_spmd

    if "nc" not in _cache:
        _cache["nc"] = _build()
    nc = _cache["nc"]

    fm_src = np.ascontiguousarray(np.asarray(fm_source, np.float32).reshape(C_FM, HW))
    fm_refm = np.ascontiguousarray(np.asarray(fm_reference, np.float32).reshape(C_FM, HW))
    m_src = np.ascontiguousarray(np.asarray(mask_source, np.float32).reshape(NP, HW))
    m_ref = np.ascontiguousarray(np.asarray(mask_ref, np.float32).reshape(NP, HW))
    r_src = np.ascontiguousarray(np.asarray(rel_pos_source, np.float32).reshape(NP, C_REL, HW))
    r_ref = np.ascontiguousarray(np.asarray(rel_pos_ref, np.float32).reshape(NP, C_REL, HW))
    w_l = np.ascontiguousarray(np.asarray(w_lambda, np.float32).reshape(1, C_FM))
    w_b = np.ascontiguousarray(np.asarray(w_beta, np.float32).reshape(1, C_FM))

    in_maps = []
    for k in range(NCORES):
        sl = slice(k * S, (k + 1) * S)
        in_maps.append({
            "fm_src_sl": np.ascontiguousarray(fm_src[:, sl]),
            "mask_src_sl": np.ascontiguousarray(m_src[:, sl]),
            "rel_src_sl": np.ascontiguousarray(r_src[:, :, sl]),
            "fm_ref": fm_refm,
            "mask_ref": m_ref,
            "rel_ref": r_ref,
            "w_lambda": w_l,
            "w_beta": w_b,
        })

    res = run_bass_kernel_spmd(nc, in_maps, list(range(NCORES)))
    _cache["last_result"] = res

    out = np.concatenate([res.results[k]["out"] for k in range(NCORES)], axis=1)
    return out.reshape(1, C_FM, H, W).astype(np.float32)


# revision 11
# speedup vs baseline: 1.0603x; 1.0174x over previous
"""Trainium2 Bass kernel for nn_AMM_66829691126233 (sparse_attention).

Computation (see reference):
  theta = concat([fm_source*mask_source*0.01, rel_pos_source], ch).reshape(3, 392, 4096)
  phi   = concat([fm_reference*mask_ref*0.01, rel_pos_ref], ch).reshape(3, 392, 4096)
  scores = theta^T @ phi                      (3, 4096, 4096)
  attn = softmax(scores*200, -1) * (scores != 0)
  g3 = (w_lambda . fm_reference) * mask_ref   (3, 4096)
  b3 = (w_beta   . fm_reference) * mask_ref
  gamma = sum_b attn[b] @ g3[b];  beta likewise   (4096,)
  out = fm_source * (1 + gamma) + beta        (1, 256, 64, 64)

Sharding: embarrassingly parallel over query rows; core k owns pixels
[512k, 512(k+1)). Flash-style fused softmax+weighted sums; the 3x4096x4096
score matrix never leaves PSUM. No collectives.

The (scores != 0) mask is a numerical no-op (exact zeros sit >=7000 logits
below the row max after the x200 scale; their softmax weight underflows to 0).

v2 layout: the phi-side hi/lo-split tensors are built per (part, key-half)
with bufs=2 tile rotation, and prep for step s+1 is emitted before the
matmul tiles of step s, so DMA/DVE prep overlaps the PE main loop and the
PE never idles long enough for HAM to re-throttle. The two weighted sums
are split by column between DVE and GPSIMD (K_SG).

Precision scheme (unchanged from v1): rel channels as bf16 hi/lo packed
pairs; score contribution = hi.hi + hi.lo + lo.hi over chunks c2..c5:
  c2[0:128] = (th_hi[0:128],  ph_hi[0:128])
  c3[0:96]  = (th_hi[0:96],   ph_lo[0:96]);  c3[96:104] = (th_hi[128:136], ph_hi[128:136])
  c4[0:64]  = (th_lo[0:64],   ph_hi[0:64]);  c4[64:104] = (th_hi[96:136],  ph_lo[96:136])
  c5[0:72]  = (th_lo[64:136], ph_hi[64:136])
fm chunks (x0.01, masked) ride as plain bf16.
"""

import sys

if "/opt/trn_rl_repo" not in sys.path:
    sys.path.insert(0, "/opt/trn_rl_repo")

import os as _os

import numpy as np

H = W = 64
HW = H * W          # 4096
C_FM = 256
C_REL = 136
NP = 3
NCORES = 8
S = HW // NCORES    # 512 query pixels per core
NT = S // 128       # 4 query row-tiles per part
MH = 2048           # key-dim span per psum tile (half of HW)
NSUB = MH // 512    # 512-wide psum banks per span
NH = HW // MH       # key spans (halves) per row-tile

TTR2 = _os.environ.get("K_TTR2", "0") == "1"  # tensor_tensor_reduce (BROKEN on hw)
GSUB = _os.environ.get("K_GSUB", "0") == "1"  # hi/lo subtracts on gpsimd (slower: port contention)
GWGB = _os.environ.get("K_GWGB", "0") == "1"  # w_g/w_b mask mults on gpsimd (slower)
BYP = _os.environ.get("K_BYP", "0") == "1"    # stt op0=bypass variant

_cache = {}


def _build(debug=False):
    import concourse.bass as bass
    import concourse.tile as tile
    from concourse import bacc, mybir
    from concourse.masks import make_identity

    f32 = mybir.dt.float32
    f16 = mybir.dt.float16
    bf16 = mybir.dt.bfloat16
    AF = mybir.ActivationFunctionType
    ALU = mybir.AluOpType

    nc = bacc.Bacc(None, target_bir_lowering=False, debug=debug)

    fm_src_sl = nc.declare_dram_parameter("fm_src_sl", [C_FM, S], f32, isOutput=False)
    mask_src_sl = nc.declare_dram_parameter("mask_src_sl", [NP, S], f32, isOutput=False)
    rel_src_sl = nc.declare_dram_parameter("rel_src_sl", [NP, C_REL, S], f32, isOutput=False)
    fm_ref = nc.declare_dram_parameter("fm_ref", [C_FM, HW], f32, isOutput=False)
    mask_ref = nc.declare_dram_parameter("mask_ref", [NP, HW], f32, isOutput=False)
    rel_ref = nc.declare_dram_parameter("rel_ref", [NP, C_REL, HW], f32, isOutput=False)
    w_lambda = nc.declare_dram_parameter("w_lambda", [1, C_FM], f32, isOutput=False)
    w_beta = nc.declare_dram_parameter("w_beta", [1, C_FM], f32, isOutput=False)
    out_ext = nc.declare_dram_parameter("out", [C_FM, S], f32, isOutput=True)

    gb_dram = nc.dram_tensor("gb_scratch", [8, 128], f32)

    with tile.TileContext(nc) as tc:
        with (
            tc.tile_pool(name="static", bufs=1) as st,
            tc.tile_pool(name="perpart", bufs=2) as pp,
            tc.tile_pool(name="perhalf", bufs=2) as hh,
            tc.tile_pool(name="work", bufs=1) as wk,
            tc.tile_pool(name="stats", bufs=1) as sp,
            tc.tile_pool(name="psum", bufs=2, space="PSUM") as pm,
            tc.tile_pool(name="dram", bufs=2, space="DRAM") as dp,
        ):
            # ---------------- phase 0: weights, fm prep ----------------
            wlam_row = st.tile([1, C_FM], f32)
            nc.sync.dma_start(out=wlam_row[:], in_=w_lambda[0:1, :])
            wbeta_row = st.tile([1, C_FM], f32)
            nc.sync.dma_start(out=wbeta_row[:], in_=w_beta[0:1, :])
            ones1 = st.tile([1, 128], f32)
            nc.vector.memset(ones1[:], 1.0)

            fmr_bf = []
            for c in range(2):
                t = st.tile([128, HW], bf16, name=f"fmr_bf{c}")
                nc.gpsimd.dma_start(out=t[:], in_=fm_ref[c * 128:(c + 1) * 128, :])
                fmr_bf.append(t)
            fms_raw = []
            fms_bf = []
            for c in range(2):
                t = st.tile([128, S], f32, name=f"fms_raw{c}")
                nc.sync.dma_start(out=t[:], in_=fm_src_sl[c * 128:(c + 1) * 128, :])
                fms_raw.append(t)
                tb = st.tile([128, S], bf16, name=f"fms_bf{c}")
                nc.gpsimd.dma_start(out=tb[:], in_=fm_src_sl[c * 128:(c + 1) * 128, :])
                fms_bf.append(tb)

            # replicate w_lambda / w_beta chunks across partitions via K=1 matmul
            wrep_bf = []  # [wl0, wl1, wb0, wb1]
            for q, (row, c) in enumerate([(wlam_row, 0), (wlam_row, 1), (wbeta_row, 0), (wbeta_row, 1)]):
                ps_w = pm.tile([128, MH], f32, tag="ps", name=f"ps_w{q}")
                nc.tensor.matmul(
                    ps_w[:, 0:128],
                    row[0:1, c * 128:(c + 1) * 128],
                    ones1[0:1, :],
                    start=True, stop=True,
                )
                t = st.tile([128, 128], bf16, name=f"wrep{q}")
                nc.scalar.copy(t[:], ps_w[:, 0:128])
                wrep_bf.append(t)

            # old_gamma / old_beta replicated on all 128 partitions: [128, HW] bf16
            old_rep = []
            for vi in range(2):
                dst = st.tile([128, HW], bf16, name=f"old_rep{vi}")
                for hhh in range(NH):
                    pg = pm.tile([128, MH], f32, tag="ps", name=f"ps_old{vi}{hhh}")
                    for k in range(NSUB):
                        col = slice(k * 512, (k + 1) * 512)
                        src = slice(hhh * MH + k * 512, hhh * MH + (k + 1) * 512)
                        for c in range(2):
                            nc.tensor.matmul(
                                pg[:, col],
                                wrep_bf[2 * vi + c][:],
                                fmr_bf[c][:, src],
                                start=(c == 0), stop=(c == 1),
                            )
                    nc.scalar.copy(dst[:, hhh * MH:(hhh + 1) * MH], pg[:])
                old_rep.append(dst)

            # scale fm chunks by 0.01 in place (raw bf16 copies are dead after
            # the old_gamma/old_beta matmuls above)
            for c in range(2):
                nc.vector.tensor_scalar_mul(fmr_bf[c][:], fmr_bf[c][:], 0.01)
                nc.vector.tensor_scalar_mul(fms_bf[c][:], fms_bf[c][:], 0.01)
            fmr01, fms01 = fmr_bf, fms_bf

            ident = st.tile([128, 128], f32)
            make_identity(nc, ident[:])

            # persistent per-part stats (col = nt*NH + h)
            stats = []
            for b in range(NP):
                stats.append({
                    k: sp.tile([128, NT * NH], f32, tag=f"{k}{b}", name=f"{k}{b}")
                    for k in ("nm", "z", "ng", "nb")
                })

            # shared junk output for the accumulating weighted sums
            junk_d = wk.tile([128, MH], bf16, tag="junk_d", name="junk_d")

            # ---------------- prep emitters ----------------
            TP = {}  # per-part theta-side tensors
            PH = {}  # per-(part, half) phi-side tensors

            def part_prep(b):
                """theta-side (query) tensors + masks for part b."""
                d = {}
                mrep = pp.tile([128, HW], bf16, tag="mask_rep", name=f"mask_rep{b}")
                nc.gpsimd.dma_start(out=mrep[:], in_=mask_ref[b:b + 1, :].to_broadcast([128, HW]))
                d["mask"] = mrep
                msrc = pp.tile([128, S], bf16, tag="msrc_rep", name=f"msrc_rep{b}")
                nc.gpsimd.dma_start(out=msrc[:], in_=mask_src_sl[b:b + 1, :].to_broadcast([128, S]))

                raw_t = pp.tile([128, S + 32], f32, tag="raw_tax", bufs=1, name=f"raw_tax{b}")
                nc.sync.dma_start(out=raw_t[:, 0:S], in_=rel_src_sl[b, 0:128, :])
                nc.sync.dma_start(out=raw_t[:, S:S + 32], in_=rel_src_sl[b, 128:C_REL, :])
                T2x = pp.tile([128, S + 32], bf16, tag="T2x", name=f"T2x{b}")
                nc.gpsimd.dma_start(out=T2x[:, 0:S], in_=rel_src_sl[b, 0:128, :])
                nc.gpsimd.dma_start(out=T2x[:, S:S + 32], in_=rel_src_sl[b, 128:C_REL, :])

                sub_eng = nc.gpsimd if GSUB else nc.vector
                T4 = pp.tile([128, S], bf16, tag="T4", name=f"T4_{b}")
                sub_eng.tensor_tensor(out=T4[:, :], in0=raw_t[:, 0:S], in1=T2x[:, 0:S],
                                      op=ALU.subtract)
                ttailf = pp.tile([128, 32], bf16, tag="ttailf", bufs=1, name=f"ttailf{b}")
                sub_eng.tensor_tensor(out=ttailf[:], in0=raw_t[:, S:S + 32],
                                      in1=T2x[:, S:S + 32], op=ALU.subtract)
                ttaild = dp.tile([8, S], bf16, tag="ttaild", name=f"ttaild{b}")
                nc.sync.dma_start(out=ttaild[:], in_=ttailf[:])
                T5 = pp.tile([128, S], bf16, tag="T5", name=f"T5_{b}")
                nc.sync.dma_start(out=T5[0:64, :], in_=T4[64:128, :])
                nc.sync.dma_start(out=T4[64:96, :], in_=T2x[96:128, 0:S])
                nc.gpsimd.dma_start(out=T4[96:104, :], in_=rel_src_sl[b, 128:C_REL, :])
                nc.sync.dma_start(out=T5[64:72, :], in_=ttaild[:])
                T3 = pp.tile([128, S], bf16, tag="T3", name=f"T3_{b}")
                nc.sync.dma_start(out=T3[0:96, :], in_=T2x[0:96, 0:S])
                nc.gpsimd.dma_start(out=T3[96:104, :], in_=rel_src_sl[b, 128:C_REL, :])

                th_fm = []
                for c in range(2):
                    t = pp.tile([128, S], bf16, tag=f"th_fm{c}", name=f"th_fm{b}{c}")
                    nc.vector.tensor_tensor(out=t[:], in0=fms01[c][:], in1=msrc[:], op=ALU.mult)
                    th_fm.append(t)
                d.update(T2x=T2x, T3=T3, T4=T4, T5=T5, th_fm=th_fm)
                TP[b] = d

            def half_prep(b, h):
                """phi-side (key) tensors for part b, key-half h."""
                hsl = slice(h * MH, (h + 1) * MH)
                mrep = TP[b]["mask"]
                raw = hh.tile([128, MH + 128], f32, tag="raw_ax", bufs=1, name=f"raw{b}{h}")
                nc.sync.dma_start(out=raw[:, 0:MH], in_=rel_ref[b, 0:128, hsl])
                nc.sync.dma_start(out=raw[:, MH:MH + 128], in_=rel_ref[b, 128:C_REL, hsl])
                P2x = hh.tile([128, MH], bf16, tag="P2x", name=f"P2x{b}{h}")
                nc.gpsimd.dma_start(out=P2x[:], in_=rel_ref[b, 0:128, hsl])
                P2t = hh.tile([128, 128], bf16, tag="P2t", name=f"P2t{b}{h}")
                nc.gpsimd.dma_start(out=P2t[:], in_=rel_ref[b, 128:C_REL, hsl])

                sub_eng = nc.gpsimd if GSUB else nc.vector
                P3 = hh.tile([128, MH], bf16, tag="P3", name=f"P3_{b}{h}")
                sub_eng.tensor_tensor(out=P3[:, :], in0=raw[:, 0:MH], in1=P2x[:], op=ALU.subtract)
                tailf = hh.tile([128, 128], bf16, tag="tailf", bufs=1, name=f"tailf{b}{h}")
                sub_eng.tensor_tensor(out=tailf[:], in0=raw[:, MH:MH + 128], in1=P2t[:],
                                      op=ALU.subtract)
                taild = dp.tile([8, MH], bf16, tag="taild", name=f"taild{b}{h}")
                nc.sync.dma_start(out=taild[:], in_=tailf[:])
                P4 = hh.tile([128, MH], bf16, tag="P4", name=f"P4_{b}{h}")
                nc.sync.dma_start(out=P4[64:96, :], in_=P3[96:128, :])
                nc.gpsimd.dma_start(out=P3[96:104, :], in_=rel_ref[b, 128:C_REL, hsl])
                nc.sync.dma_start(out=P4[96:104, :], in_=taild[:])
                nc.sync.dma_start(out=P4[0:64, :], in_=P2x[0:64, :])
                P5 = hh.tile([128, MH], bf16, tag="P5", name=f"P5_{b}{h}")
                nc.sync.dma_start(out=P5[0:64, :], in_=P2x[64:128, :])
                nc.gpsimd.dma_start(out=P5[64:72, :], in_=rel_ref[b, 128:C_REL, hsl])

                ph_fm = []
                for c in range(2):
                    t = hh.tile([128, MH], bf16, tag=f"ph_fm{c}", name=f"ph_fm{b}{h}{c}")
                    nc.vector.tensor_tensor(out=t[:], in0=fmr01[c][:, hsl], in1=mrep[:, hsl],
                                            op=ALU.mult)
                    ph_fm.append(t)
                w_eng = nc.gpsimd if GWGB else nc.vector
                w_g = hh.tile([128, MH], bf16, tag="w_g", name=f"w_g{b}{h}")
                w_eng.tensor_tensor(out=w_g[:], in0=old_rep[0][:, hsl], in1=mrep[:, hsl],
                                    op=ALU.mult)
                w_b = hh.tile([128, MH], bf16, tag="w_b", name=f"w_b{b}{h}")
                w_eng.tensor_tensor(out=w_b[:], in0=old_rep[1][:, hsl], in1=mrep[:, hsl],
                                    op=ALU.mult)
                PH[(b, h)] = dict(P2x=P2x, P3=P3, P4=P4, P5=P5, ph_fm=ph_fm, w_g=w_g, w_b=w_b)

            # ---------------- main tile (two stages, software-pipelined) ----------------
            def tile_stage1(b, h, nt):
                """matmuls + (-200*s) fp16 copy + max tree -> nm. Returns psum+e ctx."""
                t = TP[b]
                p = PH[(b, h)]
                stt = stats[b]
                nsl = slice(nt * 128, (nt + 1) * 128)
                col = nt * NH + h
                ps = pm.tile([128, MH], f32, tag="ps", name=f"ps{b}{h}{nt}")
                chunks = [
                    (t["T2x"][0:128, nsl], p["P2x"], 128),
                    (t["T3"][0:104, nsl], p["P3"], 104),
                    (t["T4"][0:104, nsl], p["P4"], 104),
                    (t["T5"][0:72, nsl], p["P5"], 72),
                    (t["th_fm"][0][:, nsl], p["ph_fm"][0], 128),
                    (t["th_fm"][1][:, nsl], p["ph_fm"][1], 128),
                ]
                nchunks = len(chunks)
                for ci, (lhsT, ph, rows) in enumerate(chunks):
                    for k in range(NSUB):
                        pcol = slice(k * 512, (k + 1) * 512)
                        nc.tensor.matmul(ps[:, pcol], lhsT, ph[0:rows, pcol],
                                         start=(ci == 0), stop=(ci == nchunks - 1))

                # nm = min(-200*s) = -200*max(s) via fp16 copy (ulp(13000)=8 logits
                # below overflow; exp arg stays within ~8 of 0: safe in fp32)
                s2 = wk.tile([128, MH], f16, tag="s2", name=f"s2_{b}{h}{nt}", bufs=2)
                nc.scalar.mul(s2[:], ps[:], -200.0)
                m1 = wk.tile([128, MH // 2], f16, tag="m1", name=f"m1_{b}{h}{nt}", bufs=2)
                nc.vector.tensor_tensor(out=m1[:], in0=s2[:, 0:MH // 2], in1=s2[:, MH // 2:MH],
                                        op=ALU.min)
                m2 = wk.tile([128, MH // 4], f16, tag="m2", name=f"m2_{b}{h}{nt}", bufs=1)
                nc.vector.tensor_tensor(out=m2[:], in0=m1[:, 0:MH // 4], in1=m1[:, MH // 4:MH // 2],
                                        op=ALU.min)
                m3 = wk.tile([128, MH // 8], f16, tag="m3", name=f"m3_{b}{h}{nt}", bufs=1)
                nc.vector.tensor_tensor(out=m3[:], in0=m2[:, 0:MH // 8], in1=m2[:, MH // 8:MH // 4],
                                        op=ALU.min)
                nc.vector.tensor_reduce(out=stt["nm"][:, col:col + 1], in_=m3[:],
                                        axis=mybir.AxisListType.X, op=ALU.min)
                return (b, h, nt, ps)

            def tile_stage2(ctx):
                """exp + weighted sums for a tile whose stage1 already ran."""
                b, h, nt, ps = ctx
                p = PH[(b, h)]
                stt = stats[b]
                col = nt * NH + h
                e_t = wk.tile([128, MH], bf16, tag="e", name=f"e{b}{h}{nt}", bufs=2)
                nc.scalar.activation(
                    out=e_t[:], in_=ps[:], func=AF.Exp,
                    bias=stt["nm"][:, col:col + 1], scale=200.0,
                    accum_out=stt["z"][:, col:col + 1],
                )
                for key, wvec in (("g", p["w_g"]), ("b", p["w_b"])):
                    if BYP:
                        nc.vector.scalar_tensor_tensor(
                            out=junk_d[:], in0=e_t[:], scalar=0.0,
                            in1=wvec[:], op0=ALU.bypass, op1=ALU.mult,
                            accum_out=stt[f"n{key}"][:, col:col + 1],
                        )
                    else:
                        nc.vector.scalar_tensor_tensor(
                            out=junk_d[:], in0=e_t[:], scalar=1.0,
                            in1=wvec[:], op0=ALU.mult, op1=ALU.mult,
                            accum_out=stt[f"n{key}"][:, col:col + 1],
                        )

            # ---------------- schedule ----------------
            steps = [(b, h) for b in range(NP) for h in range(NH)]
            part_prep(0)
            half_prep(0, 0)
            half_prep(0, 1)
            tiles = [(b, h, nt) for (b, h) in steps for nt in range(NT)]
            pending = None  # stage2 of the previous tile, emitted one tile late
            for ti, (b, h, nt) in enumerate(tiles):
                ctx = tile_stage1(b, h, nt)
                if pending is not None:
                    tile_stage2(pending)
                pending = ctx
                # emit prep two halves ahead, at the end of each (b,h) group
                if nt == NT - 1:
                    si = steps.index((b, h))
                    ni = si + 2
                    if ni < len(steps):
                        nb_, nh_ = steps[ni]
                        if nh_ == 0:
                            part_prep(nb_)
                        half_prep(nb_, nh_)
            tile_stage2(pending)

            # ---------------- epilogue: combine stats, assemble output ----------------
            gacc = st.tile([128, NP * NT], f32)
            bacc_t = st.tile([128, NP * NT], f32)
            for b in range(NP):
                stt = stats[b]
                nm2 = stt["nm"][:].rearrange("p (t h) -> p t h", h=NH)
                nmm = sp.tile([128, NT], f32, tag=f"nmm{b}", name=f"nmm{b}")
                nc.vector.tensor_reduce(out=nmm[:], in_=nm2, axis=mybir.AxisListType.X, op=ALU.min)
                d2 = sp.tile([128, NT, NH], f32, tag=f"d2{b}", name=f"d2{b}")
                for h in range(NH):
                    nc.vector.tensor_tensor(out=d2[:, :, h], in0=nmm[:], in1=nm2[:, :, h],
                                            op=ALU.subtract)
                c2 = sp.tile([128, NT, NH], f32, tag=f"c2{b}", name=f"c2{b}")
                nc.scalar.activation(out=c2[:], in_=d2[:], func=AF.Exp)
                for name, s1, acc in (("z", "z", None),
                                      ("g", "ng", gacc),
                                      ("bb", "nb", bacc_t)):
                    tot = stt[s1]
                    sc = sp.tile([128, NT, NH], f32, tag=f"sc_{name}{b}", name=f"sc_{name}{b}")
                    nc.vector.tensor_tensor(out=sc[:], in0=tot[:].rearrange("p (t h) -> p t h", h=NH),
                                            in1=c2[:], op=ALU.mult)
                    if name == "z":
                        zi = sp.tile([128, NT], f32, tag=f"zi{b}", name=f"zi{b}")
                        nc.vector.tensor_reduce(out=zi[:], in_=sc[:], axis=mybir.AxisListType.X,
                                                op=ALU.add)
                        rz = sp.tile([128, NT], f32, tag=f"rz{b}", name=f"rz{b}")
                        nc.vector.reciprocal(rz[:], zi[:])
                    else:
                        si_t = sp.tile([128, NT], f32, tag=f"si_{name}{b}", name=f"si_{name}{b}")
                        nc.vector.tensor_reduce(out=si_t[:], in_=sc[:], axis=mybir.AxisListType.X,
                                                op=ALU.add)
                        nc.vector.tensor_tensor(out=acc[:, b * NT:(b + 1) * NT], in0=si_t[:],
                                                in1=rz[:], op=ALU.mult)

            gb_sb = st.tile([128, 8], f32)
            nc.vector.tensor_reduce(out=gb_sb[:, 0:NT],
                                    in_=gacc[:].rearrange("p (b t) -> p t b", b=NP),
                                    axis=mybir.AxisListType.X, op=ALU.add)
            nc.vector.tensor_scalar_add(gb_sb[:, 0:NT], gb_sb[:, 0:NT], 1.0)
            nc.vector.tensor_reduce(out=gb_sb[:, NT:8],
                                    in_=bacc_t[:].rearrange("p (b t) -> p t b", b=NP),
                                    axis=mybir.AxisListType.X, op=ALU.add)

            ps_t = pm.tile([128, MH], f32, tag="ps", name="ps_tr")
            nc.tensor.transpose(ps_t[:8, 0:128], gb_sb[:], ident[:])
            gb_t = st.tile([8, 128], f32)
            nc.scalar.copy(gb_t[:], ps_t[:8, 0:128])
            nc.sync.dma_start(out=gb_dram[:], in_=gb_t[:])

            g1_rep = st.tile([128, S], f32)
            nc.sync.dma_start(out=g1_rep[:],
                              in_=gb_dram[0:NT, :].unsqueeze(0).to_broadcast([128, NT, 128]))
            b_rep = st.tile([128, S], f32)
            nc.sync.dma_start(out=b_rep[:],
                              in_=gb_dram[NT:8, :].unsqueeze(0).to_broadcast([128, NT, 128]))

            for c in range(2):
                o_t = wk.tile([128, S], f32, tag="o", name=f"o{c}", bufs=2)
                nc.vector.tensor_tensor(out=o_t[:], in0=fms_raw[c][:], in1=g1_rep[:], op=ALU.mult)
                nc.vector.tensor_tensor(out=o_t[:], in0=o_t[:], in1=b_rep[:], op=ALU.add)
                nc.sync.dma_start(out=out_ext[c * 128:(c + 1) * 128, :], in_=o_t[:])

    nc.compile()
    return nc


def kernel(fm_source, fm_reference, mask_source, mask_ref,
           rel_pos_source, rel_pos_ref, w_lambda, w_beta):
    from concourse.bass_utils import run_bass_kernel_spmd

    if "nc" not in _cache:
        _cache["nc"] = _build()
    nc = _cache["nc"]

    fm_src = np.ascontiguousarray(np.asarray(fm_source, np.float32).reshape(C_FM, HW))
    fm_refm = np.ascontiguousarray(np.asarray(fm_reference, np.float32).reshape(C_FM, HW))
    m_src = np.ascontiguousarray(np.asarray(mask_source, np.float32).reshape(NP, HW))
    m_ref = np.ascontiguousarray(np.asarray(mask_ref, np.float32).reshape(NP, HW))
    r_src = np.ascontiguousarray(np.asarray(rel_pos_source, np.float32).reshape(NP, C_REL, HW))
    r_ref = np.ascontiguousarray(np.asarray(rel_pos_ref, np.float32).reshape(NP, C_REL, HW))
    w_l = np.ascontiguousarray(np.asarray(w_lambda, np.float32).reshape(1, C_FM))
    w_b = np.ascontiguousarray(np.asarray(w_beta, np.float32).reshape(1, C_FM))

    in_maps = []
    for k in range(NCORES):
        sl = slice(k * S, (k + 1) * S)
        in_maps.append({
            "fm_src_sl": np.ascontiguousarray(fm_src[:, sl]),
            "mask_src_sl": np.ascontiguousarray(m_src[:, sl]),
            "rel_src_sl": np.ascontiguousarray(r_src[:, :, sl]),
            "fm_ref": fm_refm,
            "mask_ref": m_ref,
            "rel_ref": r_ref,
            "w_lambda": w_l,
            "w_beta": w_b,
        })

    res = run_bass_kernel_spmd(nc, in_maps, list(range(NCORES)))
    _cache["last_result"] = res

    out = np.concatenate([res.results[k]["out"] for k in range(NCORES)], axis=1)
    return out.reshape(1, C_FM, H, W).astype(np.float32)
